# revision 4
# baseline (speedup 1.0000x reference)
"""MoE dispatch/combine kernel for Trainium2 (8 NeuronCores, SPMD).

Problem: out[t] = sum_{k in top2(softmax(x[t] @ Wr^T + rb))} (x[t] @ W[k] + b[k])
Shapes:  x [4,2048,1024] f32, W [8,1024,1024], b [8,1024], Wr [8,1024], rb [8].

Strategy: token-parallel. Each of the 8 cores takes 1024 tokens and all 8
experts (weights replicated). Per core:
  - transpose x once via PE (xT: [H, T] layout, H on partitions)
  - router logits via matmul (xT tiles stationary), top-2 mask via DVE max8
    + is_equal against the two largest values (softmax is monotonic, so
    top-2 of logits == top-2 of softmax scores)
  - per expert e: y_e = x @ W[e] accumulated over K in PSUM, then
    acc += mask[:, e] * y_e (ACT scale-copy + DVE add); bias enters as a
    tiny maskT @ b matmul that initializes acc.
No collectives needed; host concatenates the 8 output shards.
"""

import os
import sys

sys.path.insert(0, "/opt/trn_rl_repo")

import numpy as np

import concourse.bass as bass
import concourse.mybir as mybir
import concourse.tile as tile
from concourse.bass import ts
from concourse.masks import make_identity

F32 = mybir.dt.float32

# Per-core problem geometry (hardcoded for this problem).
B, S, H = 4, 2048, 1024
T_FULL = B * S              # 8192 tokens
N_CORES = 8
TP = T_FULL // N_CORES      # 1024 tokens per core
E = 8                       # experts
P = 128                     # partitions
KC = H // P                 # 8 contraction chunks
MC = TP // P                # 8 token chunks per core
NB = 512                    # fp32 PSUM bank free size
NN = H // NB                # 2 output halves


def build_nc() -> bass.Bass:
    nc = bass.Bass()

    x_d = nc.dram_tensor("x", [TP, H], F32, kind="ExternalInput")
    w_d = nc.dram_tensor("w", [E, H, H], F32, kind="ExternalInput")
    b_d = nc.dram_tensor("b", [E, H], F32, kind="ExternalInput")
    wr_d = nc.dram_tensor("wr", [E, H], F32, kind="ExternalInput")
    rb_d = nc.dram_tensor("rb", [1, E], F32, kind="ExternalInput")
    out_d = nc.dram_tensor("out", [TP, H], F32, kind="ExternalOutput")

    with tile.TileContext(nc) as tc:
        with (
            tc.tile_pool(name="persist", bufs=1) as persist,
            tc.tile_pool(name="xin", bufs=3) as xin,
            tc.tile_pool(name="wpool", bufs=2) as wpool,
            tc.tile_pool(name="tmp", bufs=3) as tmppool,
            tc.tile_pool(name="small", bufs=2) as small,
            tc.tile_pool(name="pt", bufs=2, space="PSUM") as pt,
            tc.tile_pool(name="pr", bufs=2, space="PSUM") as pr,
            tc.tile_pool(name="py", bufs=2, space="PSUM") as py,
        ):
            # ---- persistent tiles ----
            xT = persist.tile([P, KC, TP], F32, tag="xT")        # x^T  [h, t]
            acc = persist.tile([P, MC, H], F32, tag="acc")       # out  [t, h]
            mask = persist.tile([P, MC, E], F32, tag="mask")     # [t-chunked, e]
            maskT = persist.tile([E, TP], F32, tag="maskT")      # [e, t]
            wrT = persist.tile([P, KC, E], F32, tag="wrT")       # Wr^T [h, e]
            ident = persist.tile([P, P], F32, tag="ident")
            ones = persist.tile([1, P], F32, tag="ones")
            b_sb = persist.tile([E, H], F32, tag="b_sb")
            rb_sb = persist.tile([1, E], F32, tag="rb_sb")
            wr_sb = persist.tile([E, H], F32, tag="wr_sb")

            make_identity(nc, ident)
            nc.vector.memset(ones, 1.0)
            nc.sync.dma_start(out=b_sb, in_=b_d[:, :])
            nc.sync.dma_start(out=rb_sb, in_=rb_d[:, :])
            nc.sync.dma_start(out=wr_sb, in_=wr_d[:, :])

            # ---- Wr^T via PE transpose: [8, 128] -> [128, 8] per K chunk ----
            for k in range(KC):
                pwr = pr.tile([P, E], F32, tag="pr")
                nc.tensor.transpose(pwr, wr_sb[0:E, ts(k, P)], ident[0:E, 0:E])
                nc.any.tensor_copy(wrT[:, k, :], pwr)

            # ---- load x, transpose to xT ----
            for m in range(MC):
                x_m = xin.tile([P, H], F32, tag="x_m")
                nc.sync.dma_start(out=x_m, in_=x_d[ts(m, P), :])
                for k in range(KC):
                    ptt = pt.tile([P, P], F32, tag="pt")
                    nc.tensor.transpose(ptt, x_m[:, ts(k, P)], ident)
                    nc.any.tensor_copy(xT[:, k, ts(m, P)], ptt)

            # ---- router: logits -> top-2 mask ----
            for m in range(MC):
                plg = pr.tile([P, E], F32, tag="pr")
                for k in range(KC):
                    nc.tensor.matmul(
                        plg, xT[:, k, ts(m, P)], wrT[:, k, :],
                        start=(k == 0), stop=False,
                    )
                # router bias via K=1 matmul (ones^T @ rb)
                nc.tensor.matmul(plg, ones[0:1, :], rb_sb[0:1, :],
                                 start=False, stop=True)
                logits = small.tile([P, E], F32, tag="logits")
                nc.vector.tensor_copy(logits, plg)
                maxes = small.tile([P, E], F32, tag="maxes")
                nc.vector.max(out=maxes, in_=logits)
                m1 = small.tile([P, E], F32, tag="m1")
                nc.vector.tensor_scalar(m1, logits, maxes[:, 0:1], None,
                                        mybir.AluOpType.is_equal)
                m2 = small.tile([P, E], F32, tag="m2")
                nc.vector.tensor_scalar(m2, logits, maxes[:, 1:2], None,
                                        mybir.AluOpType.is_equal)
                nc.vector.tensor_add(mask[:, m, :], m1, m2)
                # duplicate-max tie safety: clamp to 1
                nc.vector.tensor_scalar_min(mask[:, m, :], mask[:, m, :], 1.0)
                # mask^T for the bias matmul
                pmt = pt.tile([P, P], F32, tag="pt")
                nc.tensor.transpose(pmt[0:E, :], mask[:, m, :], ident)
                nc.any.tensor_copy(maskT[0:E, ts(m, P)], pmt[0:E, :])

            # ---- init acc with bias term: acc[t] = mask[t] @ b ----
            for m in range(MC):
                pyb = py.tile([P, H], F32, tag="py")
                for n in range(NN):
                    nc.tensor.matmul(
                        pyb[:, ts(n, NB)],
                        maskT[0:E, ts(m, P)], b_sb[0:E, ts(n, NB)],
                        start=True, stop=True,
                    )
                nc.scalar.activation(acc[:, m, :], pyb,
                                     mybir.ActivationFunctionType.Copy)

            # ---- main loop: experts ----
            for e in range(E):
                we = wpool.tile([P, KC, H], F32, tag="we")
                for k in range(KC):
                    nc.sync.dma_start(out=we[:, k, :], in_=w_d[e, ts(k, P), :])
                for m in range(MC):
                    pyt = py.tile([P, H], F32, tag="py")
                    for k in range(KC):
                        for n in range(NN):
                            nc.tensor.matmul(
                                pyt[:, ts(n, NB)],
                                xT[:, k, ts(m, P)],
                                we[:, k, ts(n, NB)],
                                start=(k == 0), stop=(k == KC - 1),
                            )
                    tmp = tmppool.tile([P, H], F32, tag="tmp")
                    nc.scalar.activation(tmp, pyt,
                                         mybir.ActivationFunctionType.Copy,
                                         scale=mask[:, m, e:e + 1])
                    nc.any.tensor_add(acc[:, m, :], acc[:, m, :], tmp)

            # ---- store ----
            for m in range(MC):
                nc.sync.dma_start(out=out_d[ts(m, P), :], in_=acc[:, m, :])

    return nc


MAX_WAITS = 1


def _split_sync_waits(bir_bytes: bytes, max_waits: int = MAX_WAITS) -> bytes:
    """Cap per-instruction on_wait count for the installed walrus.

    The walrus codegen in this image rejects TPB_CTRL instructions with
    more than `max_waits` sem-wait conditions ("Too many sync wait
    commands"). Tile's kernel-tail drain waits on every live semaphore on
    one instruction. Splitting the waits across preceding same-engine
    NoOps preserves semantics (engine streams execute in order, so
    serial waits AND together).
    """
    import json

    m = json.loads(bir_bytes)
    uid = [0]
    for fn in m["functions"]:
        for bb in fn["blocks"]:
            new_insts = []
            for inst in bb["instructions"]:
                si = inst.get("sync_info")
                waits = (si or {}).get("on_wait") or []
                if len(waits) > max_waits:
                    extra = waits[:-max_waits]
                    inst["sync_info"]["on_wait"] = waits[-max_waits:]
                    for ci in range(0, len(extra), max_waits):
                        chunk = extra[ci:ci + max_waits]
                        uid[0] += 1
                        new_insts.append({
                            "debug": inst.get("debug", 0),
                            "engine": inst["engine"],
                            "ins": [],
                            "name": f"{inst['name']}-wsplit{uid[0]}",
                            "opcode": "NoOp",
                            "outs": [],
                            "sync_info": {"on_update": [], "on_wait": chunk},
                        })
                new_insts.append(inst)
            bb["instructions"] = new_insts
    return json.dumps(m).encode()


_NC = None


def _get_nc():
    global _NC
    if _NC is None:
        nc = build_nc()
        patched = _split_sync_waits(nc.to_json_bytes())
        nc.to_json_bytes = lambda: patched
        _NC = nc
    return _NC


def _run(in_maps, trace=False):
    from concourse.bass_utils import run_bass_kernel_spmd

    nc = _get_nc()
    return run_bass_kernel_spmd(nc, in_maps, list(range(N_CORES)), trace=trace)


def make_in_maps(hidden_states, weight, bias, router_weight, router_bias):
    flat = np.ascontiguousarray(
        np.asarray(hidden_states, dtype=np.float32).reshape(T_FULL, H))
    w = np.ascontiguousarray(np.asarray(weight, dtype=np.float32))
    b = np.ascontiguousarray(np.asarray(bias, dtype=np.float32))
    wr = np.ascontiguousarray(np.asarray(router_weight, dtype=np.float32))
    rb = np.ascontiguousarray(
        np.asarray(router_bias, dtype=np.float32).reshape(1, E))
    return [
        {"x": flat[c * TP:(c + 1) * TP], "w": w, "b": b, "wr": wr, "rb": rb}
        for c in range(N_CORES)
    ]


def kernel(hidden_states, weight, bias, router_weight, router_bias):
    in_maps = make_in_maps(hidden_states, weight, bias, router_weight,
                           router_bias)
    res = _run(in_maps, trace=False)
    out = np.concatenate([res.results[c]["out"] for c in range(N_CORES)],
                         axis=0)
    return out.reshape(B, S, H).astype(np.float32)


# revision 5
# speedup vs baseline: 1.0010x; 1.0010x over previous
"""MoE dispatch/combine kernel for Trainium2 (8 NeuronCores, SPMD).

Problem: out[t] = sum_{k in top2(softmax(x[t] @ Wr^T + rb))} (x[t] @ W[k] + b[k])
Shapes:  x [4,2048,1024] f32, W [8,1024,1024], b [8,1024], Wr [8,1024], rb [8].

Strategy: token-parallel. Each of the 8 cores takes 1024 tokens and all 8
experts (weights replicated). Per core:
  - transpose x once via PE (xT: [H, T] layout, H on partitions)
  - router logits via matmul (xT tiles stationary), top-2 mask via DVE max8
    + is_equal against the two largest values (softmax is monotonic, so
    top-2 of logits == top-2 of softmax scores)
  - per expert e: y_e = x @ W[e] accumulated over K in PSUM, then
    acc += mask[:, e] * y_e (ACT scale-copy + DVE add); bias enters as a
    tiny maskT @ b matmul that initializes acc.
No collectives needed; host concatenates the 8 output shards.
"""

import os
import sys

sys.path.insert(0, "/opt/trn_rl_repo")

import numpy as np

import concourse.bass as bass
import concourse.mybir as mybir
import concourse.tile as tile
from concourse.bass import ts
from concourse.masks import make_identity

F32 = mybir.dt.float32

# Per-core problem geometry (hardcoded for this problem).
B, S, H = 4, 2048, 1024
T_FULL = B * S              # 8192 tokens
N_CORES = 8
TP = T_FULL // N_CORES      # 1024 tokens per core
E = 8                       # experts
P = 128                     # partitions
KC = H // P                 # 8 contraction chunks
MC = TP // P                # 8 token chunks per core
NB = 512                    # fp32 PSUM bank free size
NN = H // NB                # 2 output halves


def build_nc() -> bass.Bass:
    nc = bass.Bass()

    x_d = nc.dram_tensor("x", [TP, H], F32, kind="ExternalInput")
    w_d = nc.dram_tensor("w", [E, H, H], F32, kind="ExternalInput")
    b_d = nc.dram_tensor("b", [E, H], F32, kind="ExternalInput")
    wr_d = nc.dram_tensor("wr", [E, H], F32, kind="ExternalInput")
    rb_d = nc.dram_tensor("rb", [1, E], F32, kind="ExternalInput")
    out_d = nc.dram_tensor("out", [TP, H], F32, kind="ExternalOutput")

    with tile.TileContext(nc) as tc:
        with (
            tc.tile_pool(name="persist", bufs=1) as persist,
            tc.tile_pool(name="xin", bufs=3) as xin,
            tc.tile_pool(name="wpool", bufs=2) as wpool,
            tc.tile_pool(name="tmp", bufs=3) as tmppool,
            tc.tile_pool(name="small", bufs=2) as small,
            tc.tile_pool(name="pt", bufs=2, space="PSUM") as pt,
            tc.tile_pool(name="pr", bufs=2, space="PSUM") as pr,
            tc.tile_pool(name="py", bufs=2, space="PSUM") as py,
        ):
            # ---- persistent tiles ----
            xT = persist.tile([P, KC, TP], F32, tag="xT")        # x^T  [h, t]
            acc = persist.tile([P, MC, H], F32, tag="acc")       # out  [t, h]
            mask = persist.tile([P, MC, E], F32, tag="mask")     # [t-chunked, e]
            maskT = persist.tile([E, TP], F32, tag="maskT")      # [e, t]
            wrT = persist.tile([P, KC, E], F32, tag="wrT")       # Wr^T [h, e]
            ident = persist.tile([P, P], F32, tag="ident")
            ones = persist.tile([1, P], F32, tag="ones")
            b_sb = persist.tile([E, H], F32, tag="b_sb")
            rb_sb = persist.tile([1, E], F32, tag="rb_sb")
            wr_sb = persist.tile([E, H], F32, tag="wr_sb")

            make_identity(nc, ident)
            nc.vector.memset(ones, 1.0)
            nc.sync.dma_start(out=b_sb, in_=b_d[:, :])
            nc.sync.dma_start(out=rb_sb, in_=rb_d[:, :])
            nc.sync.dma_start(out=wr_sb, in_=wr_d[:, :])

            # ---- Wr^T via PE transpose: [8, 128] -> [128, 8] per K chunk ----
            for k in range(KC):
                pwr = pr.tile([P, E], F32, tag="pr")
                nc.tensor.transpose(pwr, wr_sb[0:E, ts(k, P)], ident[0:E, 0:E])
                nc.any.tensor_copy(wrT[:, k, :], pwr)

            # ---- load x, transpose to xT ----
            for m in range(MC):
                x_m = xin.tile([P, H], F32, tag="x_m")
                nc.sync.dma_start(out=x_m, in_=x_d[ts(m, P), :])
                for k in range(KC):
                    ptt = pt.tile([P, P], F32, tag="pt")
                    nc.tensor.transpose(ptt, x_m[:, ts(k, P)], ident)
                    nc.any.tensor_copy(xT[:, k, ts(m, P)], ptt)

            # ---- router: logits -> top-2 mask ----
            for m in range(MC):
                plg = pr.tile([P, E], F32, tag="pr")
                for k in range(KC):
                    nc.tensor.matmul(
                        plg, xT[:, k, ts(m, P)], wrT[:, k, :],
                        start=(k == 0), stop=False,
                    )
                # router bias via K=1 matmul (ones^T @ rb)
                nc.tensor.matmul(plg, ones[0:1, :], rb_sb[0:1, :],
                                 start=False, stop=True)
                logits = small.tile([P, E], F32, tag="logits")
                nc.vector.tensor_copy(logits, plg)
                maxes = small.tile([P, E], F32, tag="maxes")
                nc.vector.max(out=maxes, in_=logits)
                m1 = small.tile([P, E], F32, tag="m1")
                nc.vector.tensor_scalar(m1, logits, maxes[:, 0:1], None,
                                        mybir.AluOpType.is_equal)
                m2 = small.tile([P, E], F32, tag="m2")
                nc.vector.tensor_scalar(m2, logits, maxes[:, 1:2], None,
                                        mybir.AluOpType.is_equal)
                nc.vector.tensor_add(mask[:, m, :], m1, m2)
                # duplicate-max tie safety: clamp to 1
                nc.vector.tensor_scalar_min(mask[:, m, :], mask[:, m, :], 1.0)
                # mask^T for the bias matmul
                pmt = pt.tile([P, P], F32, tag="pt")
                nc.tensor.transpose(pmt[0:E, :], mask[:, m, :], ident)
                nc.any.tensor_copy(maskT[0:E, ts(m, P)], pmt[0:E, :])

            # ---- init acc with bias term: acc[t] = mask[t] @ b ----
            for m in range(MC):
                pyb = py.tile([P, H], F32, tag="py")
                for n in range(NN):
                    nc.tensor.matmul(
                        pyb[:, ts(n, NB)],
                        maskT[0:E, ts(m, P)], b_sb[0:E, ts(n, NB)],
                        start=True, stop=True,
                    )
                nc.scalar.activation(acc[:, m, :], pyb,
                                     mybir.ActivationFunctionType.Copy)

            # ---- main loop: experts ----
            for e in range(E):
                we = wpool.tile([P, KC, H], F32, tag="we")
                for k in range(KC):
                    nc.sync.dma_start(out=we[:, k, :], in_=w_d[e, ts(k, P), :])
                for m in range(MC):
                    pyt = py.tile([P, H], F32, tag="py")
                    for k in range(KC):
                        for n in range(NN):
                            nc.tensor.matmul(
                                pyt[:, ts(n, NB)],
                                xT[:, k, ts(m, P)],
                                we[:, k, ts(n, NB)],
                                start=(k == 0), stop=(k == KC - 1),
                            )
                    tmp = tmppool.tile([P, H], F32, tag="tmp")
                    nc.scalar.activation(tmp, pyt,
                                         mybir.ActivationFunctionType.Copy,
                                         scale=mask[:, m, e:e + 1])
                    nc.any.tensor_add(acc[:, m, :], acc[:, m, :], tmp)

            # ---- store ----
            for m in range(MC):
                nc.sync.dma_start(out=out_d[ts(m, P), :], in_=acc[:, m, :])

    return nc


MAX_WAITS = 1


def _split_sync_waits(bir_bytes: bytes, max_waits: int = MAX_WAITS) -> bytes:
    """Cap per-instruction on_wait count for the installed walrus.

    The walrus codegen in this image rejects TPB_CTRL instructions with
    more than `max_waits` sem-wait conditions ("Too many sync wait
    commands"). Tile's kernel-tail drain waits on every live semaphore on
    one instruction. Splitting the waits across preceding same-engine
    NoOps preserves semantics (engine streams execute in order, so
    serial waits AND together).
    """
    import json

    m = json.loads(bir_bytes)
    uid = [0]
    for fn in m["functions"]:
        for bb in fn["blocks"]:
            new_insts = []
            for inst in bb["instructions"]:
                si = inst.get("sync_info")
                waits = (si or {}).get("on_wait") or []
                if len(waits) > max_waits:
                    extra = waits[:-max_waits]
                    inst["sync_info"]["on_wait"] = waits[-max_waits:]
                    for ci in range(0, len(extra), max_waits):
                        chunk = extra[ci:ci + max_waits]
                        uid[0] += 1
                        new_insts.append({
                            "debug": inst.get("debug", 0),
                            "engine": inst["engine"],
                            "ins": [],
                            "name": f"{inst['name']}-wsplit{uid[0]}",
                            "opcode": "NoOp",
                            "outs": [],
                            "sync_info": {"on_update": [], "on_wait": chunk},
                        })
                new_insts.append(inst)
            bb["instructions"] = new_insts
    return json.dumps(m).encode()


_NC = None


def _get_nc():
    global _NC
    if _NC is None:
        nc = build_nc()
        patched = _split_sync_waits(nc.to_json_bytes())
        nc.to_json_bytes = lambda: patched
        _NC = nc
    return _NC


def _run(in_maps, trace=False, **kwargs):
    from concourse.bass_utils import run_bass_kernel_spmd

    nc = _get_nc()
    return run_bass_kernel_spmd(nc, in_maps, list(range(N_CORES)), trace=trace,
                                **kwargs)


def make_in_maps(hidden_states, weight, bias, router_weight, router_bias):
    flat = np.ascontiguousarray(
        np.asarray(hidden_states, dtype=np.float32).reshape(T_FULL, H))
    w = np.ascontiguousarray(np.asarray(weight, dtype=np.float32))
    b = np.ascontiguousarray(np.asarray(bias, dtype=np.float32))
    wr = np.ascontiguousarray(np.asarray(router_weight, dtype=np.float32))
    rb = np.ascontiguousarray(
        np.asarray(router_bias, dtype=np.float32).reshape(1, E))
    return [
        {"x": flat[c * TP:(c + 1) * TP], "w": w, "b": b, "wr": wr, "rb": rb}
        for c in range(N_CORES)
    ]


def kernel(hidden_states, weight, bias, router_weight, router_bias):
    in_maps = make_in_maps(hidden_states, weight, bias, router_weight,
                           router_bias)
    res = _run(in_maps, trace=False)
    out = np.concatenate([res.results[c]["out"] for c in range(N_CORES)],
                         axis=0)
    return out.reshape(B, S, H).astype(np.float32)
